# revision 13
# baseline (speedup 1.0000x reference)
"""Trainium2 Bass kernel for nn_AttentionMechanism (batched attention with
per-sample queries), data-parallel across 8 NeuronCores.

Math (per batch row b):
    q = msgs @ Wq.T + bq                         [H]
    k_t = Wk @ tau_t + bk ; scores_t = q.k_t/32
    alpha = softmax(scores) ; out = sum_t alpha_t (Wv @ tau_t + bv)

Rewrite used (exact up to softmax shift invariance):
    qk   = msgs @ (Wq.T @ Wk) + bq @ Wk          [TAU]   (q.bk const in t -> cancels)
    scores_t = qk . tau_t / 32
    p_t  = exp(scores_t)            (scores are O(1), no max-subtraction needed)
    ctx  = sum_t p_t tau_t / sum_t p_t
    out  = (ctx @ Wv.T) / sum_t p_t + bv         (normalization deferred to the
                                                  [128, VDIM] output)

Host precomputes the batch-independent weight products (Wfused = Wq.T @ Wk,
qk_bias = bq @ Wk, WvT = Wv.T) and packs them bf16, so the device streams tau
once from HBM (32 MB/core) plus ~1.5 MB of weights. The tau stream paces the
kernel (~90 us at 358 GB/s); all compute is sized to hide under it.

Device schedule per 128-row b-tile, per t-chunk of up to 8 trajectory steps:
  DMA   : chunk [128, tc, 1024] f32->bf16 cast on load
  Vector: mult prod = chunk * qk_rep in 2 halves; 3D tensor_reduce for the
          second half's scores
  Scalar: activation-accum scores for the first half; exp; diag builds
          (diag_t = ident * p_t via activation scale)
  PE    : 2*tc matmuls ctx[:, bank] += diag(p_t) @ chunk_t  (p_t scaling
          folded into the matmul weights)
The last b-tile tapers its chunks (8,8,8,4,2,2) to shorten the compute tail
after the final DMA byte lands.
"""

import math

import numpy as np
import ml_dtypes

import concourse.bass as bass
import concourse.bacc as bacc
import concourse.tile as tile
from concourse import mybir
from concourse.bass_utils import run_bass_kernel_spmd
from concourse.masks import make_identity

F32 = mybir.dt.float32
BF16 = mybir.dt.bfloat16
NP_BF16 = ml_dtypes.bfloat16

B = 2048
T = 32
TAU = 1024
MSG = 512
HID = 1024
VDIM = 128
N_CORES = 8
B_LOCAL = B // N_CORES

Alu = mybir.AluOpType
Act = mybir.ActivationFunctionType

TCM = 8  # max t_chunk


def build(b_local=B_LOCAL, chunk_bufs=5, prod_bufs=3):
    assert b_local % 128 == 0
    n_btiles = b_local // 128

    # per-btile chunk schedules; the last btile tapers to shorten the tail
    schedules = [[TCM] * (T // TCM) for _ in range(n_btiles)]
    schedules[-1] = [8, 8, 8, 4, 2, 2]
    assert all(sum(s) == T for s in schedules)

    nc = bacc.Bacc("TRN2", target_bir_lowering=False, debug=False)

    traj = nc.declare_dram_parameter(
        "imagined_trajectory", [b_local, T * TAU], F32, isOutput=False
    )
    msgsT = nc.declare_dram_parameter("msgsT", [MSG, b_local], BF16, isOutput=False)
    Wfused = nc.declare_dram_parameter("Wfused", [MSG, TAU], BF16, isOutput=False)
    qkbias = nc.declare_dram_parameter("qkbias", [TAU], BF16, isOutput=False)
    WvT = nc.declare_dram_parameter("WvT", [TAU, VDIM], BF16, isOutput=False)
    bv = nc.declare_dram_parameter("bv", [VDIM], F32, isOutput=False)
    out = nc.declare_dram_parameter("out", [b_local, VDIM], F32, isOutput=True)

    MQ = MSG // 128  # 4 m-chunks
    CQ = TAU // 128  # 8 c-chunks

    with tile.TileContext(nc) as tc:
        with (
            tc.tile_pool(name="const", bufs=1) as const,
            tc.tile_pool(name="persist", bufs=1) as persist,
            tc.tile_pool(name="psum_big", bufs=2, space="PSUM") as psum_big,
            tc.tile_pool(name="psum_tr", bufs=2, space="PSUM") as psum_tr,
            tc.tile_pool(name="psum_out", bufs=2, space="PSUM") as psum_out,
        ):
            # weights for qk on gpsimd (ring DMA), ahead of the tau stream
            Wfused_b = const.tile([128, MQ, TAU], BF16)
            nc.gpsimd.dma_start(
                out=Wfused_b, in_=Wfused[:, :].rearrange("(j p) c -> p j c", p=128)
            )
            msgsT_b = const.tile([128, MQ, b_local], BF16)
            nc.gpsimd.dma_start(
                out=msgsT_b, in_=msgsT[:, :].rearrange("(j p) b -> p j b", p=128)
            )
            qkb_sb = const.tile([1, TAU], BF16)
            nc.gpsimd.dma_start(out=qkb_sb, in_=qkbias[None, :])
            # epilogue-only weights on the sync queue (not needed until ~60us)
            WvT_b = const.tile([128, CQ, VDIM], BF16)
            nc.sync.dma_start(
                out=WvT_b, in_=WvT[:, :].rearrange("(j p) d -> p j d", p=128)
            )
            bv_sb = const.tile([1, VDIM], F32)
            nc.sync.dma_start(out=bv_sb, in_=bv[None, :])

            ident_f = const.tile([128, 128], F32)
            make_identity(nc, ident_f)
            ident_b = const.tile([128, 128], BF16)
            make_identity(nc, ident_b)
            onespad_b = const.tile([128, 128], BF16)
            nc.vector.memset(onespad_b, 0.0)
            nc.vector.memset(onespad_b[0:1, :], 1.0)
            bvpad_b = const.tile([128, VDIM], BF16)
            nc.vector.memset(bvpad_b, 0.0)
            nc.vector.tensor_copy(out=bvpad_b[0:1, :], in_=bv_sb)
            ones_row = const.tile([1, b_local], BF16)
            nc.vector.memset(ones_row, 1.0)
            # bv broadcast to all 128 partitions (for the final bias add)
            pbv = psum_out.tile([128, VDIM], F32, tag="mm", name="pbv")
            nc.tensor.matmul(pbv, lhsT=onespad_b, rhs=bvpad_b, start=True, stop=True)
            bv_bcast = const.tile([128, VDIM], F32)
            nc.vector.tensor_copy(out=bv_bcast, in_=pbv)

            # qk[b, c] = (msgs @ Wfused + qk_bias) / sqrt(H)
            qk_b = [
                persist.tile([128, TAU], BF16, tag=f"qkb{i}", name=f"qkb{i}")
                for i in range(n_btiles)
            ]
            for bi in range(n_btiles):
                bsl = slice(bi * 128, (bi + 1) * 128)
                pq = psum_big.tile([128, TAU], F32, tag="ctx", name="pq")
                for nh in range(2):
                    nsl = slice(nh * 512, (nh + 1) * 512)
                    for mi in range(MQ):
                        nc.tensor.matmul(
                            pq[:, nsl],
                            lhsT=msgsT_b[:, mi, bsl],
                            rhs=Wfused_b[:, mi, nsl],
                            start=(mi == 0),
                            stop=False,
                        )
                    nc.tensor.matmul(
                        pq[:, nsl],
                        lhsT=ones_row[:, bsl],
                        rhs=qkb_sb[:, nsl],
                        start=False,
                        stop=True,
                    )
                nc.scalar.mul(out=qk_b[bi], in_=pq, mul=1.0 / math.sqrt(HID))

            # ---------- main loop: stream tau ----------
            with (
                tc.tile_pool(name="stream", bufs=chunk_bufs) as stream,
                tc.tile_pool(name="bfp", bufs=prod_bufs) as bfp,
                tc.tile_pool(name="dpool", bufs=2) as dpool,
                tc.tile_pool(name="spool", bufs=4) as spool,
                tc.tile_pool(name="aux", bufs=2) as aux,
            ):
                dumm = aux.tile([128, TAU], BF16, tag="dumm", name="dumm", bufs=1)
                for bi in range(n_btiles):
                    sched = schedules[bi]
                    n_chunks = len(sched)
                    starts = [sum(sched[:i]) for i in range(n_chunks)]
                    bsl = slice(bi * 128, (bi + 1) * 128)
                    ctx_ps = psum_big.tile([128, TAU], F32, tag="ctx", name="ctx_ps")
                    p_all = aux.tile([128, T], F32, tag="p", name="p_all")

                    # issue all chunk DMAs for this b-tile up front
                    chunks = []
                    for ci in range(n_chunks):
                        tcn = sched[ci]
                        chunk_bf = stream.tile(
                            [128, TCM, TAU], BF16, tag="chunk", name="chunk_bf"
                        )
                        c0 = starts[ci] * TAU
                        nc.gpsimd.dma_start(
                            out=chunk_bf[:, :tcn, :],
                            in_=traj[bsl, c0 : c0 + tcn * TAU].rearrange(
                                "p (t c) -> p t c", t=tcn
                            ),
                        )
                        chunks.append(chunk_bf)

                    for ci in range(n_chunks):
                        tcn = sched[ci]
                        chunk_bf = chunks[ci]
                        c0t = starts[ci]
                        h1 = max(1, tcn // 2)  # scalar-reduced slices
                        h2 = tcn - h1  # vector-reduced slices
                        # prod = chunk * qk (broadcast over t), in two halves
                        # so scalar accums can start after the first half
                        prod = bfp.tile(
                            [128, TCM, TAU], BF16, tag="prod", name="prod"
                        )
                        for lo, hi in ((0, h1), (h1, tcn)):
                            if hi <= lo:
                                continue
                            qk_rep = bass.AP(
                                tensor=qk_b[bi].tensor,
                                offset=qk_b[bi].offset,
                                ap=[qk_b[bi].ap[0], [0, hi - lo], [1, TAU]],
                            )
                            nc.vector.tensor_tensor(
                                out=prod[:, lo:hi, :],
                                in0=chunk_bf[:, lo:hi, :],
                                in1=qk_rep,
                                op=Alu.mult,
                            )
                        scores_sc = spool.tile(
                            [128, h1], F32, tag="ssc", name="scores_sc"
                        )
                        for tt in range(h1):
                            nc.scalar.activation(
                                out=dumm,
                                in_=prod[:, tt, :],
                                func=Act.Copy,
                                accum_out=scores_sc[:, tt : tt + 1],
                            )
                        if h2 > 0:
                            scores_ve = spool.tile(
                                [128, h2], F32, tag="sve", name="scores_ve"
                            )
                            nc.vector.tensor_reduce(
                                out=scores_ve,
                                in_=prod[:, h1:tcn, :],
                                axis=mybir.AxisListType.X,
                                op=Alu.add,
                            )
                        nc.scalar.activation(
                            out=p_all[:, c0t : c0t + h1],
                            in_=scores_sc,
                            func=Act.Exp,
                        )
                        if h2 > 0:
                            nc.scalar.activation(
                                out=p_all[:, c0t + h1 : c0t + tcn],
                                in_=scores_ve,
                                func=Act.Exp,
                            )
                        # diag blocks + matmuls, pipelined per t-slice
                        diag_all = dpool.tile(
                            [128, TCM, 128], BF16, tag="diag", name="diag_all"
                        )
                        for tt in range(tcn):
                            col = c0t + tt
                            nc.scalar.activation(
                                out=diag_all[:, tt, :],
                                in_=ident_b,
                                func=Act.Copy,
                                scale=p_all[:, col : col + 1],
                            )
                            first = ci == 0 and tt == 0
                            last = ci == n_chunks - 1 and tt == tcn - 1
                            for nh in range(2):
                                nc.tensor.matmul(
                                    ctx_ps[:, nh * 512 : (nh + 1) * 512],
                                    lhsT=diag_all[:, tt, :],
                                    rhs=chunk_bf[:, tt, nh * 512 : (nh + 1) * 512],
                                    start=first,
                                    stop=last,
                                )

                    # epilogue: out = (ctx @ Wv.T) / sum_t p_t + bv
                    s_sum = aux.tile([128, 1], F32, tag="ssum", name="s_sum")
                    nc.vector.tensor_reduce(
                        out=s_sum, in_=p_all, axis=mybir.AxisListType.X, op=Alu.add
                    )
                    rinv = aux.tile([128, 1], F32, tag="rinv", name="rinv")
                    nc.vector.reciprocal(out=rinv, in_=s_sum)
                    ctxc_b = aux.tile([128, TAU], BF16, tag="ctxc", name="ctxc_b")
                    nc.vector.tensor_copy(out=ctxc_b, in_=ctx_ps)
                    ctxT_b = aux.tile([128, CQ, 128], BF16, tag="ctxT", name="ctxT_b")
                    for j in range(CQ):
                        ptb = psum_tr.tile([128, 128], BF16, tag="tr", name="ptb")
                        nc.tensor.transpose(
                            ptb, ctxc_b[:, j * 128 : (j + 1) * 128], ident_b
                        )
                        if j % 2 == 0:
                            nc.scalar.copy(out=ctxT_b[:, j, :], in_=ptb)
                        else:
                            nc.vector.tensor_copy(out=ctxT_b[:, j, :], in_=ptb)
                    pm = psum_out.tile([128, VDIM], F32, tag="mm", name="pm")
                    for j in range(CQ):
                        nc.tensor.matmul(
                            pm,
                            lhsT=ctxT_b[:, j, :],
                            rhs=WvT_b[:, j, :],
                            start=(j == 0),
                            stop=(j == CQ - 1),
                        )
                    msg_s = aux.tile([128, VDIM], F32, tag="msgs", name="msg_s")
                    nc.scalar.activation(
                        out=msg_s, in_=pm, func=Act.Copy, scale=rinv
                    )
                    msg_out = aux.tile([128, VDIM], F32, tag="msg", name="msg_out")
                    nc.vector.tensor_tensor(
                        out=msg_out, in0=msg_s, in1=bv_bcast, op=Alu.add
                    )
                    nc.sync.dma_start(out=out[bsl, :], in_=msg_out)

    nc.compile()
    return nc


_NC_CACHE = {}


def _get_nc():
    key = "default"
    if key not in _NC_CACHE:
        _NC_CACHE[key] = build()
    return _NC_CACHE[key]


def make_in_maps(imagined_trajectory, received_messages, Wq, bq, Wk, Wv, bv):
    Wq = np.asarray(Wq, dtype=np.float32)
    bq = np.asarray(bq, dtype=np.float32)
    Wk = np.asarray(Wk, dtype=np.float32)
    Wv = np.asarray(Wv, dtype=np.float32)
    bv = np.asarray(bv, dtype=np.float32)
    # batch-independent weight fusion, done once on host
    Wfused = np.ascontiguousarray(Wq.T @ Wk).astype(NP_BF16)  # [MSG, TAU]
    qkbias = (bq @ Wk).astype(NP_BF16)  # [TAU]
    WvT = np.ascontiguousarray(Wv.T).astype(NP_BF16)  # [TAU, VDIM]

    bl = B_LOCAL
    in_maps = []
    for i in range(N_CORES):
        sl = slice(i * bl, (i + 1) * bl)
        in_maps.append(
            {
                "imagined_trajectory": np.ascontiguousarray(
                    imagined_trajectory[sl], dtype=np.float32
                ),
                "msgsT": np.ascontiguousarray(
                    np.asarray(received_messages[sl], dtype=np.float32).T
                ).astype(NP_BF16),
                "Wfused": Wfused,
                "qkbias": qkbias,
                "WvT": WvT,
                "bv": bv,
            }
        )
    return in_maps


def kernel(
    imagined_trajectory,
    received_messages,
    Wq,
    bq,
    Wk,
    bk,
    Wv,
    bv,
):
    nc = _get_nc()
    in_maps = make_in_maps(
        imagined_trajectory, received_messages, Wq, bq, Wk, Wv, bv
    )
    res = run_bass_kernel_spmd(nc, in_maps, list(range(N_CORES)))
    return np.concatenate([res.results[i]["out"] for i in range(N_CORES)], axis=0)


# revision 14
# speedup vs baseline: 1.2100x; 1.2100x over previous
"""Trainium2 Bass kernel for nn_AttentionMechanism (batched attention with
per-sample queries), data-parallel across 8 NeuronCores.

Math (per batch row b):
    q = msgs @ Wq.T + bq                         [H]
    k_t = Wk @ tau_t + bk ; scores_t = q.k_t/32
    alpha = softmax(scores) ; out = sum_t alpha_t (Wv @ tau_t + bv)

Rewrite used (exact up to softmax shift invariance):
    qk   = msgs @ (Wq.T @ Wk) + bq @ Wk          [TAU]   (q.bk const in t -> cancels)
    scores_t = qk . tau_t / 32
    p_t  = exp(scores_t)            (scores are O(1), no max-subtraction needed)
    ctx  = sum_t p_t tau_t / sum_t p_t
    out  = (ctx @ Wv.T) / sum_t p_t + bv         (normalization deferred to the
                                                  [128, VDIM] output)

Host precomputes the batch-independent weight products (Wfused = Wq.T @ Wk,
qk_bias = bq @ Wk, WvT = Wv.T) and packs them bf16, so the device streams tau
once from HBM (32 MB/core) plus ~1.5 MB of weights. The tau stream paces the
kernel (~90 us at 358 GB/s); all compute is sized to hide under it.

Device schedule per 128-row b-tile, per t-chunk of up to 8 trajectory steps:
  DMA   : chunk [128, tc, 1024] f32->bf16 cast on load
  Vector: mult prod = chunk * qk_rep in 2 halves; 3D tensor_reduce for the
          second half's scores
  Scalar: activation-accum scores for the first half; exp; diag builds
          (diag_t = ident * p_t via activation scale)
  PE    : 2*tc matmuls ctx[:, bank] += diag(p_t) @ chunk_t  (p_t scaling
          folded into the matmul weights)
The last b-tile tapers its chunks (8,8,8,4,2,2) to shorten the compute tail
after the final DMA byte lands.
"""

import math

import numpy as np
import ml_dtypes

import concourse.bass as bass
import concourse.bacc as bacc
import concourse.tile as tile
from concourse import mybir
from concourse.bass_utils import run_bass_kernel_spmd
from concourse.masks import make_identity

F32 = mybir.dt.float32
BF16 = mybir.dt.bfloat16
NP_BF16 = ml_dtypes.bfloat16

B = 2048
T = 32
TAU = 1024
MSG = 512
HID = 1024
VDIM = 128
N_CORES = 8
B_LOCAL = B // N_CORES

Alu = mybir.AluOpType
Act = mybir.ActivationFunctionType

TCM = 8  # max t_chunk


def build(b_local=B_LOCAL, chunk_bufs=5, prod_bufs=3):
    assert b_local % 128 == 0
    n_btiles = b_local // 128

    # per-btile chunk schedules; the last btile tapers to shorten the tail
    schedules = [[TCM] * (T // TCM) for _ in range(n_btiles)]
    schedules[-1] = [8, 8, 8, 4, 2, 2]
    assert all(sum(s) == T for s in schedules)

    nc = bacc.Bacc("TRN2", target_bir_lowering=False, debug=False)

    traj = nc.declare_dram_parameter(
        "imagined_trajectory", [b_local, T * TAU], F32, isOutput=False
    )
    msgsT = nc.declare_dram_parameter("msgsT", [MSG, b_local], BF16, isOutput=False)
    Wfused = nc.declare_dram_parameter("Wfused", [MSG, TAU], BF16, isOutput=False)
    qkbias = nc.declare_dram_parameter("qkbias", [TAU], BF16, isOutput=False)
    WvT = nc.declare_dram_parameter("WvT", [TAU, VDIM], BF16, isOutput=False)
    bv = nc.declare_dram_parameter("bv", [VDIM], F32, isOutput=False)
    out = nc.declare_dram_parameter("out", [b_local, VDIM], F32, isOutput=True)

    MQ = MSG // 128  # 4 m-chunks
    CQ = TAU // 128  # 8 c-chunks

    with tile.TileContext(nc) as tc:
        with (
            tc.tile_pool(name="const", bufs=1) as const,
            tc.tile_pool(name="persist", bufs=1) as persist,
            tc.tile_pool(name="psum_big", bufs=2, space="PSUM") as psum_big,
            tc.tile_pool(name="psum_tr", bufs=2, space="PSUM") as psum_tr,
            tc.tile_pool(name="psum_out", bufs=2, space="PSUM") as psum_out,
        ):
            # weights for qk on gpsimd (ring DMA), ahead of the tau stream
            Wfused_b = const.tile([128, MQ, TAU], BF16)
            nc.gpsimd.dma_start(
                out=Wfused_b, in_=Wfused[:, :].rearrange("(j p) c -> p j c", p=128)
            )
            msgsT_b = const.tile([128, MQ, b_local], BF16)
            nc.gpsimd.dma_start(
                out=msgsT_b, in_=msgsT[:, :].rearrange("(j p) b -> p j b", p=128)
            )
            qkb_sb = const.tile([1, TAU], BF16)
            nc.gpsimd.dma_start(out=qkb_sb, in_=qkbias[None, :])
            # epilogue-only weights on the sync queue (not needed until ~60us)
            WvT_b = const.tile([128, CQ, VDIM], BF16)
            nc.sync.dma_start(
                out=WvT_b, in_=WvT[:, :].rearrange("(j p) d -> p j d", p=128)
            )
            bv_sb = const.tile([1, VDIM], F32)
            nc.sync.dma_start(out=bv_sb, in_=bv[None, :])

            ident_f = const.tile([128, 128], F32)
            make_identity(nc, ident_f)
            ident_b = const.tile([128, 128], BF16)
            make_identity(nc, ident_b)
            onespad_b = const.tile([128, 128], BF16)
            nc.vector.memset(onespad_b, 0.0)
            nc.vector.memset(onespad_b[0:1, :], 1.0)
            bvpad_b = const.tile([128, VDIM], BF16)
            nc.vector.memset(bvpad_b, 0.0)
            nc.vector.tensor_copy(out=bvpad_b[0:1, :], in_=bv_sb)
            ones_row = const.tile([1, b_local], BF16)
            nc.vector.memset(ones_row, 1.0)
            # bv broadcast to all 128 partitions (for the final bias add)
            pbv = psum_out.tile([128, VDIM], F32, tag="mm", name="pbv")
            nc.tensor.matmul(pbv, lhsT=onespad_b, rhs=bvpad_b, start=True, stop=True)
            bv_bcast = const.tile([128, VDIM], F32)
            nc.vector.tensor_copy(out=bv_bcast, in_=pbv)

            # qk[b, c] = (msgs @ Wfused + qk_bias) / sqrt(H)
            qk_b = [
                persist.tile([128, TAU], BF16, tag=f"qkb{i}", name=f"qkb{i}")
                for i in range(n_btiles)
            ]
            for bi in range(n_btiles):
                bsl = slice(bi * 128, (bi + 1) * 128)
                pq = psum_big.tile([128, TAU], F32, tag="ctx", name="pq")
                for nh in range(2):
                    nsl = slice(nh * 512, (nh + 1) * 512)
                    for mi in range(MQ):
                        nc.tensor.matmul(
                            pq[:, nsl],
                            lhsT=msgsT_b[:, mi, bsl],
                            rhs=Wfused_b[:, mi, nsl],
                            start=(mi == 0),
                            stop=False,
                        )
                    nc.tensor.matmul(
                        pq[:, nsl],
                        lhsT=ones_row[:, bsl],
                        rhs=qkb_sb[:, nsl],
                        start=False,
                        stop=True,
                    )
                nc.scalar.mul(out=qk_b[bi], in_=pq, mul=1.0 / math.sqrt(HID))

            # ---------- main loop: stream tau ----------
            with (
                tc.tile_pool(name="stream", bufs=chunk_bufs) as stream,
                tc.tile_pool(name="bfp", bufs=prod_bufs) as bfp,
                tc.tile_pool(name="dpool", bufs=2) as dpool,
                tc.tile_pool(name="spool", bufs=4) as spool,
                tc.tile_pool(name="aux", bufs=2) as aux,
            ):
                dumm = aux.tile([128, TAU], BF16, tag="dumm", name="dumm", bufs=1)
                for bi in range(n_btiles):
                    sched = schedules[bi]
                    n_chunks = len(sched)
                    starts = [sum(sched[:i]) for i in range(n_chunks)]
                    bsl = slice(bi * 128, (bi + 1) * 128)
                    ctx_ps = psum_big.tile([128, TAU], F32, tag="ctx", name="ctx_ps")
                    p_all = aux.tile([128, T], F32, tag="p", name="p_all")

                    # issue all chunk DMAs for this b-tile up front
                    chunks = []
                    for ci in range(n_chunks):
                        tcn = sched[ci]
                        chunk_bf = stream.tile(
                            [128, TCM, TAU], BF16, tag="chunk", name="chunk_bf"
                        )
                        c0 = starts[ci] * TAU
                        nc.gpsimd.dma_start(
                            out=chunk_bf[:, :tcn, :],
                            in_=traj[bsl, c0 : c0 + tcn * TAU].rearrange(
                                "p (t c) -> p t c", t=tcn
                            ),
                        )
                        chunks.append(chunk_bf)

                    for ci in range(n_chunks):
                        tcn = sched[ci]
                        chunk_bf = chunks[ci]
                        c0t = starts[ci]
                        h1 = max(1, tcn // 2)  # scalar-reduced slices
                        h2 = tcn - h1  # vector-reduced slices
                        # prod = chunk * qk (broadcast over t) in one DVE pass
                        prod = bfp.tile(
                            [128, TCM, TAU], BF16, tag="prod", name="prod"
                        )
                        qk_rep = bass.AP(
                            tensor=qk_b[bi].tensor,
                            offset=qk_b[bi].offset,
                            ap=[qk_b[bi].ap[0], [0, tcn], [1, TAU]],
                        )
                        nc.vector.tensor_tensor(
                            out=prod[:, :tcn, :],
                            in0=chunk_bf[:, :tcn, :],
                            in1=qk_rep,
                            op=Alu.mult,
                        )
                        scores_sc = spool.tile(
                            [128, h1], F32, tag="ssc", name="scores_sc"
                        )
                        for tt in range(h1):
                            nc.scalar.activation(
                                out=dumm,
                                in_=prod[:, tt, :],
                                func=Act.Copy,
                                accum_out=scores_sc[:, tt : tt + 1],
                            )
                        if h2 > 0:
                            scores_ve = spool.tile(
                                [128, h2], F32, tag="sve", name="scores_ve"
                            )
                            nc.vector.tensor_reduce(
                                out=scores_ve,
                                in_=prod[:, h1:tcn, :],
                                axis=mybir.AxisListType.X,
                                op=Alu.add,
                            )
                        nc.scalar.activation(
                            out=p_all[:, c0t : c0t + h1],
                            in_=scores_sc,
                            func=Act.Exp,
                        )
                        if h2 > 0:
                            nc.scalar.activation(
                                out=p_all[:, c0t + h1 : c0t + tcn],
                                in_=scores_ve,
                                func=Act.Exp,
                            )
                        diag_all = dpool.tile(
                            [128, TCM, 128], BF16, tag="diag", name="diag_all"
                        )
                        # vector builds the diags for the scalar-scored slices
                        # (ready first) in one broadcast mult; scalar builds
                        # the rest per-slice. Neither blocks its own inputs.
                        ident_rep = bass.AP(
                            tensor=ident_b.tensor,
                            offset=ident_b.offset,
                            ap=[ident_b.ap[0], [0, h1], [1, 128]],
                        )
                        p_sl = p_all[:, c0t : c0t + h1]
                        p_rep = bass.AP(
                            tensor=p_sl.tensor,
                            offset=p_sl.offset,
                            ap=[p_sl.ap[0], p_sl.ap[1], [0, 128]],
                        )
                        nc.vector.tensor_tensor(
                            out=diag_all[:, :h1, :],
                            in0=ident_rep,
                            in1=p_rep,
                            op=Alu.mult,
                        )
                        for tt in range(h1, tcn):
                            col = c0t + tt
                            nc.scalar.activation(
                                out=diag_all[:, tt, :],
                                in_=ident_b,
                                func=Act.Copy,
                                scale=p_all[:, col : col + 1],
                            )
                        for tt in range(tcn):
                            first = ci == 0 and tt == 0
                            last = ci == n_chunks - 1 and tt == tcn - 1
                            for nh in range(2):
                                nc.tensor.matmul(
                                    ctx_ps[:, nh * 512 : (nh + 1) * 512],
                                    lhsT=diag_all[:, tt, :],
                                    rhs=chunk_bf[:, tt, nh * 512 : (nh + 1) * 512],
                                    start=first,
                                    stop=last,
                                )

                    # epilogue: out = (ctx @ Wv.T) / sum_t p_t + bv
                    s_sum = aux.tile([128, 1], F32, tag="ssum", name="s_sum")
                    nc.vector.tensor_reduce(
                        out=s_sum, in_=p_all, axis=mybir.AxisListType.X, op=Alu.add
                    )
                    rinv = aux.tile([128, 1], F32, tag="rinv", name="rinv")
                    nc.vector.reciprocal(out=rinv, in_=s_sum)
                    ctxc_b = aux.tile([128, TAU], BF16, tag="ctxc", name="ctxc_b")
                    nc.vector.tensor_copy(out=ctxc_b, in_=ctx_ps)
                    ctxT_b = aux.tile([128, CQ, 128], BF16, tag="ctxT", name="ctxT_b")
                    for j in range(CQ):
                        ptb = psum_tr.tile([128, 128], BF16, tag="tr", name="ptb")
                        nc.tensor.transpose(
                            ptb, ctxc_b[:, j * 128 : (j + 1) * 128], ident_b
                        )
                        if j % 2 == 0:
                            nc.scalar.copy(out=ctxT_b[:, j, :], in_=ptb)
                        else:
                            nc.vector.tensor_copy(out=ctxT_b[:, j, :], in_=ptb)
                    pm = psum_out.tile([128, VDIM], F32, tag="mm", name="pm")
                    for j in range(CQ):
                        nc.tensor.matmul(
                            pm,
                            lhsT=ctxT_b[:, j, :],
                            rhs=WvT_b[:, j, :],
                            start=(j == 0),
                            stop=(j == CQ - 1),
                        )
                    msg_s = aux.tile([128, VDIM], F32, tag="msgs", name="msg_s")
                    nc.scalar.activation(
                        out=msg_s, in_=pm, func=Act.Copy, scale=rinv
                    )
                    msg_out = aux.tile([128, VDIM], F32, tag="msg", name="msg_out")
                    nc.vector.tensor_tensor(
                        out=msg_out, in0=msg_s, in1=bv_bcast, op=Alu.add
                    )
                    nc.sync.dma_start(out=out[bsl, :], in_=msg_out)

    nc.compile()
    return nc


_NC_CACHE = {}


def _get_nc():
    key = "default"
    if key not in _NC_CACHE:
        _NC_CACHE[key] = build()
    return _NC_CACHE[key]


def make_in_maps(imagined_trajectory, received_messages, Wq, bq, Wk, Wv, bv):
    Wq = np.asarray(Wq, dtype=np.float32)
    bq = np.asarray(bq, dtype=np.float32)
    Wk = np.asarray(Wk, dtype=np.float32)
    Wv = np.asarray(Wv, dtype=np.float32)
    bv = np.asarray(bv, dtype=np.float32)
    # batch-independent weight fusion, done once on host
    Wfused = np.ascontiguousarray(Wq.T @ Wk).astype(NP_BF16)  # [MSG, TAU]
    qkbias = (bq @ Wk).astype(NP_BF16)  # [TAU]
    WvT = np.ascontiguousarray(Wv.T).astype(NP_BF16)  # [TAU, VDIM]

    bl = B_LOCAL
    in_maps = []
    for i in range(N_CORES):
        sl = slice(i * bl, (i + 1) * bl)
        in_maps.append(
            {
                "imagined_trajectory": np.ascontiguousarray(
                    imagined_trajectory[sl], dtype=np.float32
                ),
                "msgsT": np.ascontiguousarray(
                    np.asarray(received_messages[sl], dtype=np.float32).T
                ).astype(NP_BF16),
                "Wfused": Wfused,
                "qkbias": qkbias,
                "WvT": WvT,
                "bv": bv,
            }
        )
    return in_maps


def kernel(
    imagined_trajectory,
    received_messages,
    Wq,
    bq,
    Wk,
    bk,
    Wv,
    bv,
):
    nc = _get_nc()
    in_maps = make_in_maps(
        imagined_trajectory, received_messages, Wq, bq, Wk, Wv, bv
    )
    res = run_bass_kernel_spmd(nc, in_maps, list(range(N_CORES)))
    return np.concatenate([res.results[i]["out"] for i in range(N_CORES)], axis=0)
